# revision 13
# baseline (speedup 1.0000x reference)
# Trainium2 Bass kernel for nn_ManyDeepFeatureMatcher (mutual-NN ratio matcher).
#
# Sharding: core c owns rows [c*1024,(c+1)*1024) of d1 (=A columns) for the
# forward direction and rows [c*1024,(c+1)*1024) of d2 (=B columns) for the
# reverse direction. Each core normalizes full A and B on device, computes its
# sim slab [1024, 8192] via fp32 matmuls in BOTH orientations (so each
# direction's top-k reduces along the free dim), and emits per-row top-2
# values + argmax index. Host does only the tiny final mask/gather math.
import numpy as np

import concourse.bacc as bacc
import concourse.mybir as mybir
import concourse.tile as tile
from concourse.bass_utils import run_bass_kernel_spmd

C = 256          # feature dim (contraction)
N = 8192         # descriptors per map
CORES = 8
SLAB = N // CORES          # 1024 rows per core per direction
MT = SLAB // 128           # 8 m-tiles per slab
QW = 2048                  # psum quarter width (4 banks)
NQ = N // QW               # 4 quarters
NCHUNK = 512               # normalization chunk width
F32 = mybir.dt.float32

_CACHE = {}


def _build():
    nc = bacc.Bacc("TRN2", target_bir_lowering=False, debug=False, num_devices=CORES)

    a_lhs_d = nc.dram_tensor("a_lhs", [C, SLAB], F32, kind="ExternalInput")
    b_lhs_d = nc.dram_tensor("b_lhs", [C, SLAB], F32, kind="ExternalInput")

    v1_d = nc.dram_tensor("v1", [SLAB, 2], F32, kind="ExternalOutput")   # dir1 top2
    i1_d = nc.dram_tensor("i1", [SLAB, 1], F32, kind="ExternalOutput")   # dir1 argmax j
    v2_d = nc.dram_tensor("v2", [SLAB, 2], F32, kind="ExternalOutput")   # dir2 top2
    i2_d = nc.dram_tensor("i2", [SLAB, 1], F32, kind="ExternalOutput")   # dir2 argmax i

    with tile.TileContext(nc) as tc:
        with (
            tc.tile_pool(name="big", bufs=1) as big,
            tc.tile_pool(name="work", bufs=2) as work,
            tc.tile_pool(name="out", bufs=4) as outp,
            tc.tile_pool(name="ps", bufs=2, space="PSUM") as ps,
            tc.tile_pool(name="dram", bufs=1, space="DRAM") as dram,
        ):

            ones_k = big.tile([128, 1], F32)
            nc.vector.memset(ones_k[:], 1.0)
            ones_r = big.tile([1, 128], F32)
            nc.vector.memset(ones_r[:], 1.0)

            # persistent normalized tensors, one [128, *] tile per k-half
            # per-slab chunk tiles: [k-half][slab c] -> [128, SLAB]
            an = [[big.tile([128, SLAB], F32, tag=f"an{k}_{c}", name=f"an{k}_{c}")
                   for c in range(CORES)] for k in range(2)]
            bn = [[big.tile([128, SLAB], F32, tag=f"bn{k}_{c}", name=f"bn{k}_{c}")
                   for c in range(CORES)] for k in range(2)]
            an_s = [big.tile([128, SLAB], F32, tag=f"ans{k}", name=f"ans{k}") for k in range(2)]
            bn_s = [big.tile([128, SLAB], F32, tag=f"bns{k}", name=f"bns{k}") for k in range(2)]

            def normalize(src_fn, width, dst):
                # dst: list of two [128, width] tiles (k halves), L2-normalized cols
                QWN = min(width, 1024)
                for t in range(width // QWN):
                    cs = slice(t * QWN, (t + 1) * QWN)
                    ap0, ap1 = src_fn(t, QWN)
                    raw = work.tile([128, 2, QWN], F32, tag="raw")
                    nc.sync.dma_start(raw[:, 0, :], ap0)
                    nc.sync.dma_start(raw[:, 1, :], ap1)
                    sq = work.tile([128, 2, QWN], F32, tag="sq")
                    nc.scalar.square(sq[:], raw[:])
                    pn = ps.tile([1, QWN], F32, tag="ps")
                    for n in range(QWN // 512):
                        ns = slice(n * 512, (n + 1) * 512)
                        nc.tensor.matmul(pn[:, ns], ones_k[:], sq[:, 0, ns],
                                         start=True, stop=False)
                        nc.tensor.matmul(pn[:, ns], ones_k[:], sq[:, 1, ns],
                                         start=False, stop=True)
                    s_row = work.tile([1, QWN], F32, tag="s_row")
                    nc.scalar.activation(
                        s_row[:], pn[:], mybir.ActivationFunctionType.Sqrt
                    )
                    inv_row = work.tile([1, QWN], F32, tag="inv_row")
                    nc.vector.reciprocal(inv_row[:], s_row[:])
                    pr = ps.tile([128, QWN], F32, tag="ps")
                    for n in range(QWN // 512):
                        ns = slice(n * 512, (n + 1) * 512)
                        nc.tensor.matmul(pr[:, ns], ones_r[:], inv_row[:, ns],
                                         start=True, stop=True)
                    invrep = work.tile([128, QWN], F32, tag="invrep")
                    nc.scalar.copy(invrep[:], pr[:])
                    for k in range(2):
                        nc.vector.tensor_tensor(
                            out=dst[k][:, cs], in0=raw[:, k, :], in1=invrep[:],
                            op=mybir.AluOpType.mult,
                        )

            def inp_src(d):
                def fn(t, w):
                    cs = slice(t * w, (t + 1) * w)
                    return d.ap()[0:128, cs], d.ap()[128:256, cs]
                return fn

            # normalize ONLY the local slabs (1/8 of the work) ...
            normalize(inp_src(a_lhs_d), SLAB, an_s)
            normalize(inp_src(b_lhs_d), SLAB, bn_s)

            # ... then AllGather the NORMALIZED slabs: the gathered result is
            # the final normalized full tensor, loaded to SBUF by bulk DMA.
            ab_bounce = dram.tile([2, C, SLAB], F32)
            nc.sync.dma_start(ab_bounce[0, 0:128, :], an_s[0][:])
            nc.sync.dma_start(ab_bounce[0, 128:256, :], an_s[1][:])
            nc.sync.dma_start(ab_bounce[1, 0:128, :], bn_s[0][:])
            nc.sync.dma_start(ab_bounce[1, 128:256, :], bn_s[1][:])
            ab_gath = dram.tile([CORES, 2, C, SLAB], F32)
            nc.gpsimd.collective_compute(
                "AllGather", mybir.AluOpType.bypass,
                replica_groups=[list(range(CORES))],
                ins=[ab_bounce.opt()], outs=[ab_gath.opt()],
            )
            for k in range(2):
                ks = slice(k * 128, (k + 1) * 128)
                for c in range(CORES):
                    nc.sync.dma_start(an[k][c][:], ab_gath[c, 0, ks, :])
                    nc.sync.dma_start(bn[k][c][:], ab_gath[c, 1, ks, :])

            def direction(lhs_tiles, rhs_tiles, vd, idxd):
                for m in range(MT):
                    ms = slice(m * 128, (m + 1) * 128)
                    cvals = work.tile([128, NQ * 8], F32, tag="cvals")
                    cidx = work.tile([128, NQ * 8], F32, tag="cidx")
                    for q in range(NQ):
                        pq = ps.tile([128, QW], F32, tag="ps")
                        for n in range(QW // 512):
                            j0 = q * QW + n * 512
                            c, off = j0 // SLAB, j0 % SLAB
                            js = slice(off, off + 512)
                            nc.tensor.matmul(
                                pq[:, n * 512:(n + 1) * 512],
                                lhs_tiles[0][:, ms], rhs_tiles[0][c][:, js],
                                start=True, stop=False,
                            )
                            nc.tensor.matmul(
                                pq[:, n * 512:(n + 1) * 512],
                                lhs_tiles[1][:, ms], rhs_tiles[1][c][:, js],
                                start=False, stop=True,
                            )
                        qs = slice(q * 8, (q + 1) * 8)
                        nc.vector.max(out=cvals[:, qs], in_=pq[:])
                        iu = work.tile([128, 8], mybir.dt.uint32, tag="iu")
                        nc.vector.max_index(out=iu[:], in_max=cvals[:, qs], in_values=pq[:])
                        nc.vector.tensor_scalar(
                            out=cidx[:, qs], in0=iu[:], scalar1=float(q * QW),
                            scalar2=None, op0=mybir.AluOpType.add,
                        )
                    top8 = outp.tile([128, 8], F32, tag="top8")
                    nc.vector.max(out=top8[:], in_=cvals[:])
                    scratch = work.tile([128, NQ * 8], F32, tag="scratch")
                    nn_i = outp.tile([128, 1], F32, tag="nn_i")
                    nc.vector.scalar_tensor_tensor(
                        out=scratch[:], in0=cvals[:], scalar=top8[:, 0:1],
                        in1=cidx[:], op0=mybir.AluOpType.is_equal,
                        op1=mybir.AluOpType.mult, accum_out=nn_i[:],
                    )
                    nc.sync.dma_start(vd.ap()[ms, :], top8[:, 0:2])
                    nc.sync.dma_start(idxd.ap()[ms, :], nn_i[:])

            direction(an_s, bn, v1_d, i1_d)   # dir 1->2: rows=d1 slab, cols=d2
            direction(bn_s, an, v2_d, i2_d)   # dir 2->1: rows=d2 slab, cols=d1

    nc.finalize()
    return nc


def _get_nc():
    if "nc" not in _CACHE:
        _CACHE["nc"] = _build()
    return _CACHE["nc"]


RATIO = 0.9
EPS = 1e-8


def kernel(map_A, map_B):
    map_A = np.ascontiguousarray(np.asarray(map_A, dtype=np.float32))
    map_B = np.ascontiguousarray(np.asarray(map_B, dtype=np.float32))
    nc = _get_nc()
    in_maps = []
    for c in range(CORES):
        sl = slice(c * SLAB, (c + 1) * SLAB)
        in_maps.append({
            "a_lhs": np.ascontiguousarray(map_A[:, sl]),
            "b_lhs": np.ascontiguousarray(map_B[:, sl]),
        })
    globals()["_last_in_maps"] = in_maps
    res = run_bass_kernel_spmd(nc, in_maps, core_ids=list(range(CORES)))
    r = res.results

    v1 = np.concatenate([r[c]["v1"] for c in range(CORES)], axis=0)   # [N, 2]
    nn12 = np.concatenate([r[c]["i1"][:, 0] for c in range(CORES)])   # [N]
    v2 = np.concatenate([r[c]["v2"] for c in range(CORES)], axis=0)
    nn21 = np.concatenate([r[c]["i2"][:, 0] for c in range(CORES)])

    nn12 = np.clip(nn12, 0, N - 1).astype(np.int64)
    nn21 = np.clip(nn21, 0, N - 1).astype(np.int64)

    # fp32 arithmetic to mirror the reference
    d1 = np.float32(2.0) - np.float32(2.0) * v1[:, 0]
    d2_ = np.float32(2.0) - np.float32(2.0) * v1[:, 1]
    r12 = d1 / (d2_ + np.float32(EPS))
    dt1 = np.float32(2.0) - np.float32(2.0) * v2[:, 0]
    dt2 = np.float32(2.0) - np.float32(2.0) * v2[:, 1]
    r21 = dt1 / (dt2 + np.float32(EPS))

    ids1 = np.arange(N, dtype=np.int64)
    mask = (ids1 == nn21[nn12]) & (r12 <= RATIO) & (r21[nn12] <= RATIO)

    matches = np.where(
        mask[:, None],
        np.stack([ids1, nn12], axis=-1),
        -1,
    ).astype(np.int32)
    match_sim = np.where(mask, v1[:, 0], np.float32(0.0)).astype(np.float32)
    return matches, match_sim, mask.astype(np.bool_)


# revision 15
# speedup vs baseline: 1.0132x; 1.0132x over previous
# Trainium2 Bass kernel for nn_ManyDeepFeatureMatcher (mutual-NN ratio matcher).
#
# Sharding: core c owns rows [c*1024,(c+1)*1024) of d1 (=A columns) for the
# forward direction and rows [c*1024,(c+1)*1024) of d2 (=B columns) for the
# reverse direction. Each core normalizes full A and B on device, computes its
# sim slab [1024, 8192] via fp32 matmuls in BOTH orientations (so each
# direction's top-k reduces along the free dim), and emits per-row top-2
# values + argmax index. Host does only the tiny final mask/gather math.
import numpy as np

import concourse.bacc as bacc
import concourse.mybir as mybir
import concourse.tile as tile
from concourse.bass_utils import run_bass_kernel_spmd

C = 256          # feature dim (contraction)
N = 8192         # descriptors per map
CORES = 8
SLAB = N // CORES          # 1024 rows per core per direction
MT = SLAB // 128           # 8 m-tiles per slab
QW = 2048                  # psum quarter width (4 banks)
NQ = N // QW               # 4 quarters
NCHUNK = 512               # normalization chunk width
F32 = mybir.dt.float32

_CACHE = {}


def _build():
    nc = bacc.Bacc("TRN2", target_bir_lowering=False, debug=False, num_devices=CORES)

    a_lhs_d = nc.dram_tensor("a_lhs", [C, SLAB], F32, kind="ExternalInput")
    b_lhs_d = nc.dram_tensor("b_lhs", [C, SLAB], F32, kind="ExternalInput")

    v1_d = nc.dram_tensor("v1", [SLAB, 2], F32, kind="ExternalOutput")   # dir1 top2
    i1_d = nc.dram_tensor("i1", [SLAB, 1], F32, kind="ExternalOutput")   # dir1 argmax j
    v2_d = nc.dram_tensor("v2", [SLAB, 2], F32, kind="ExternalOutput")   # dir2 top2
    i2_d = nc.dram_tensor("i2", [SLAB, 1], F32, kind="ExternalOutput")   # dir2 argmax i

    with tile.TileContext(nc) as tc:
        with (
            tc.tile_pool(name="big", bufs=1) as big,
            tc.tile_pool(name="work", bufs=2) as work,
            tc.tile_pool(name="out", bufs=4) as outp,
            tc.tile_pool(name="ps", bufs=2, space="PSUM") as ps,
            tc.tile_pool(name="dram", bufs=1, space="DRAM") as dram,
        ):

            ones_k = big.tile([128, 1], F32)
            nc.vector.memset(ones_k[:], 1.0)
            ones_r = big.tile([1, 128], F32)
            nc.vector.memset(ones_r[:], 1.0)

            # persistent normalized tensors, one [128, *] tile per k-half
            # per-slab chunk tiles: [k-half][slab c] -> [128, SLAB]
            an = [[big.tile([128, SLAB], F32, tag=f"an{k}_{c}", name=f"an{k}_{c}")
                   for c in range(CORES)] for k in range(2)]
            bn = [[big.tile([128, SLAB], F32, tag=f"bn{k}_{c}", name=f"bn{k}_{c}")
                   for c in range(CORES)] for k in range(2)]
            an_s = [big.tile([128, SLAB], F32, tag=f"ans{k}", name=f"ans{k}") for k in range(2)]
            bn_s = [big.tile([128, SLAB], F32, tag=f"bns{k}", name=f"bns{k}") for k in range(2)]

            def normalize(src_fn, width, dst):
                # dst: list of two [128, width] tiles (k halves), L2-normalized cols
                QWN = min(width, 1024)
                for t in range(width // QWN):
                    cs = slice(t * QWN, (t + 1) * QWN)
                    ap0, ap1 = src_fn(t, QWN)
                    raw = work.tile([128, 2, QWN], F32, tag="raw")
                    nc.sync.dma_start(raw[:, 0, :], ap0)
                    nc.sync.dma_start(raw[:, 1, :], ap1)
                    sq = work.tile([128, 2, QWN], F32, tag="sq")
                    nc.scalar.square(sq[:], raw[:])
                    pn = ps.tile([1, QWN], F32, tag="ps")
                    for n in range(QWN // 512):
                        ns = slice(n * 512, (n + 1) * 512)
                        nc.tensor.matmul(pn[:, ns], ones_k[:], sq[:, 0, ns],
                                         start=True, stop=False)
                        nc.tensor.matmul(pn[:, ns], ones_k[:], sq[:, 1, ns],
                                         start=False, stop=True)
                    s_row = work.tile([1, QWN], F32, tag="s_row")
                    nc.scalar.activation(
                        s_row[:], pn[:], mybir.ActivationFunctionType.Sqrt
                    )
                    inv_row = work.tile([1, QWN], F32, tag="inv_row")
                    nc.vector.reciprocal(inv_row[:], s_row[:])
                    pr = ps.tile([128, QWN], F32, tag="ps")
                    for n in range(QWN // 512):
                        ns = slice(n * 512, (n + 1) * 512)
                        nc.tensor.matmul(pr[:, ns], ones_r[:], inv_row[:, ns],
                                         start=True, stop=True)
                    invrep = work.tile([128, QWN], F32, tag="invrep")
                    nc.scalar.copy(invrep[:], pr[:])
                    for k in range(2):
                        nc.vector.tensor_tensor(
                            out=dst[k][:, cs], in0=raw[:, k, :], in1=invrep[:],
                            op=mybir.AluOpType.mult,
                        )

            def inp_src(d):
                def fn(t, w):
                    cs = slice(t * w, (t + 1) * w)
                    return d.ap()[0:128, cs], d.ap()[128:256, cs]
                return fn

            # normalize ONLY the local slabs (1/8 of the work) ...
            normalize(inp_src(a_lhs_d), SLAB, an_s)
            normalize(inp_src(b_lhs_d), SLAB, bn_s)

            # ... then AllGather the NORMALIZED slabs: the gathered result is
            # the final normalized full tensor, loaded to SBUF by bulk DMA.
            ab_bounce = dram.tile([2, C, SLAB], F32)
            nc.sync.dma_start(ab_bounce[0, 0:128, :], an_s[0][:])
            nc.sync.dma_start(ab_bounce[0, 128:256, :], an_s[1][:])
            nc.sync.dma_start(ab_bounce[1, 0:128, :], bn_s[0][:])
            nc.sync.dma_start(ab_bounce[1, 128:256, :], bn_s[1][:])
            ab_gath = dram.tile([CORES, 2, C, SLAB], F32)
            nc.gpsimd.collective_compute(
                "AllGather", mybir.AluOpType.bypass,
                replica_groups=[list(range(CORES))],
                ins=[ab_bounce.opt()], outs=[ab_gath.opt()],
            )
            for k in range(2):
                ks = slice(k * 128, (k + 1) * 128)
                for c in range(CORES):
                    nc.sync.dma_start(an[k][c][:], ab_gath[c, 0, ks, :])
                    nc.sync.dma_start(bn[k][c][:], ab_gath[c, 1, ks, :])

            def direction(lhs_tiles, rhs_tiles, vd, idxd):
                for m in range(MT):
                    ms = slice(m * 128, (m + 1) * 128)
                    cvals = work.tile([128, NQ * 8], F32, tag="cvals")
                    cidx = work.tile([128, NQ * 8], F32, tag="cidx")
                    for q in range(NQ):
                        pq = ps.tile([128, QW], F32, tag="ps")
                        for n in range(QW // 512):
                            j0 = q * QW + n * 512
                            c, off = j0 // SLAB, j0 % SLAB
                            js = slice(off, off + 512)
                            nc.tensor.matmul(
                                pq[:, n * 512:(n + 1) * 512],
                                lhs_tiles[0][:, ms], rhs_tiles[0][c][:, js],
                                start=True, stop=False,
                            )
                            nc.tensor.matmul(
                                pq[:, n * 512:(n + 1) * 512],
                                lhs_tiles[1][:, ms], rhs_tiles[1][c][:, js],
                                start=False, stop=True,
                            )
                        qs = slice(q * 8, (q + 1) * 8)
                        nc.vector.max(out=cvals[:, qs], in_=pq[:])
                        iu = work.tile([128, 8], mybir.dt.uint32, tag="iu")
                        nc.vector.max_index(out=iu[:], in_max=cvals[:, qs], in_values=pq[:])
                        nc.vector.tensor_scalar(
                            out=cidx[:, qs], in0=iu[:], scalar1=float(q * QW),
                            scalar2=None, op0=mybir.AluOpType.add,
                        )
                    top8 = outp.tile([128, 8], F32, tag="top8")
                    nc.vector.max(out=top8[:], in_=cvals[:])
                    scratch = work.tile([128, NQ * 8], F32, tag="scratch")
                    nn_i = outp.tile([128, 1], F32, tag="nn_i")
                    nc.vector.scalar_tensor_tensor(
                        out=scratch[:], in0=cvals[:], scalar=top8[:, 0:1],
                        in1=cidx[:], op0=mybir.AluOpType.is_equal,
                        op1=mybir.AluOpType.mult, accum_out=nn_i[:],
                    )
                    nc.sync.dma_start(vd.ap()[ms, :], top8[:, 0:2])
                    nc.sync.dma_start(idxd.ap()[ms, :], nn_i[:])

            direction(an_s, bn, v1_d, i1_d)   # dir 1->2: rows=d1 slab, cols=d2
            direction(bn_s, an, v2_d, i2_d)   # dir 2->1: rows=d2 slab, cols=d1

    nc.finalize()
    return nc


def _get_nc():
    if "nc" not in _CACHE:
        _CACHE["nc"] = _build()
    return _CACHE["nc"]


RATIO = 0.9
EPS = 1e-8


def kernel(map_A, map_B):
    map_A = np.ascontiguousarray(np.asarray(map_A, dtype=np.float32))
    map_B = np.ascontiguousarray(np.asarray(map_B, dtype=np.float32))
    nc = _get_nc()
    in_maps = []
    for c in range(CORES):
        sl = slice(c * SLAB, (c + 1) * SLAB)
        in_maps.append({
            "a_lhs": np.ascontiguousarray(map_A[:, sl]),
            "b_lhs": np.ascontiguousarray(map_B[:, sl]),
        })
    globals()["_last_in_maps"] = in_maps
    res = run_bass_kernel_spmd(nc, in_maps, core_ids=list(range(CORES)))
    r = res.results

    v1 = np.concatenate([r[c]["v1"] for c in range(CORES)], axis=0)   # [N, 2]
    nn12 = np.concatenate([r[c]["i1"][:, 0] for c in range(CORES)])   # [N]
    v2 = np.concatenate([r[c]["v2"] for c in range(CORES)], axis=0)
    nn21 = np.concatenate([r[c]["i2"][:, 0] for c in range(CORES)])

    nn12 = np.clip(nn12, 0, N - 1).astype(np.int64)
    nn21 = np.clip(nn21, 0, N - 1).astype(np.int64)

    # fp32 arithmetic to mirror the reference
    d1 = np.float32(2.0) - np.float32(2.0) * v1[:, 0]
    d2_ = np.float32(2.0) - np.float32(2.0) * v1[:, 1]
    r12 = d1 / (d2_ + np.float32(EPS))
    dt1 = np.float32(2.0) - np.float32(2.0) * v2[:, 0]
    dt2 = np.float32(2.0) - np.float32(2.0) * v2[:, 1]
    r21 = dt1 / (dt2 + np.float32(EPS))

    ids1 = np.arange(N, dtype=np.int64)
    mask = (ids1 == nn21[nn12]) & (r12 <= RATIO) & (r21[nn12] <= RATIO)

    matches = np.where(
        mask[:, None],
        np.stack([ids1, nn12], axis=-1),
        -1,
    ).astype(np.int32)
    match_sim = np.where(mask, v1[:, 0], np.float32(0.0)).astype(np.float32)
    return matches, match_sim, mask.astype(np.bool_)
